# revision 2
# baseline (speedup 1.0000x reference)
"""Trainium2 Bass kernel for nn_CrossAttention (dense_transformer).

Data-parallel over batch B=8 across the 8 NeuronCores. Per core, a fully
software-pipelined program:

  - LN stats on DVE (bn_stats/bn_aggr), rstd via one ACT Rsqrt, apply split
    DVE/Pool, PE transposes to feature-major bf16.
  - Projections as bf16 PE matmuls (LN affine + 1/sqrt(c) folded on host),
    psum evacuated on Pool (q/k, bias add) and ACT (v, strided token-major).
  - Depthwise 3x3 conv on PE in TOKEN-major orientation: the shifted padded
    image slice is the stationary operand, the 128x128 diagonal weight block
    the moving one, so psum accumulates skip[tok, ch] directly -- no
    transposes of the skip path at all. Bias (zero for this reference) would
    ride as a rank-1 extra tap.
  - Attention transposed (S^T = k^T.T @ q^T) with exp straight out of PSUM
    on ACT; PV in token-major orientation (P^T stationary, v_aug moving,
    65-wide free dim) so x[tok, c] + rowsum land in psum with no final
    transpose. Row sums via the ones-column-in-V trick.
  - Final normalize + skip add as one scalar_tensor_tensor per (tq, head)
    on Pool, reciprocals on DVE, column-block DMA out.

Emission interleaves chunk g's attention with chunk g+1's projections and
conv so the PE stream never waits on the ACT exp stream (the two are within
~10% of each other); PSUM peaks at exactly 8 banks.
"""

import numpy as np
import ml_dtypes

import concourse.bass as bass
import concourse.mybir as mybir
import concourse.tile as tile
from concourse import bacc, bass_utils
from concourse.masks import make_identity

F32 = mybir.dt.float32
BF16 = mybir.dt.bfloat16
AF = mybir.ActivationFunctionType
OP = mybir.AluOpType

N_CORES = 8
N1 = 1024          # query tokens (= H*W = 32*32)
N2 = 1024          # key tokens
DIM = 512
NH = 8
CH = 64            # head dim
HH = 32            # H
WW = 32            # W
NTOK = N1 // 128   # 8 token tiles
NCH = DIM // 128   # 4 feature chunks
EPS = 1e-5
PW = WW + 2        # padded image width

# tap order: center first so its start=True write covers the full psum
# region before the edge taps accumulate.
TAPS = [(0, 0), (-1, -1), (-1, 0), (-1, 1), (0, -1), (0, 1),
        (1, -1), (1, 0), (1, 1)]


def _build_program(trace_sim=False, bench_iters=0, with_cb=False):
    nc = bacc.Bacc("TRN2", target_bir_lowering=False, debug=False,
                   enable_asserts=True, num_devices=N_CORES)

    q_ap = nc.dram_tensor("query", [N1, DIM], F32, kind="ExternalInput").ap()
    k_ap = nc.dram_tensor("key", [N2, DIM], F32, kind="ExternalInput").ap()
    wq_ap = nc.dram_tensor("wq", [NCH, 128, DIM], BF16, kind="ExternalInput").ap()
    wk_ap = nc.dram_tensor("wk", [NCH, 128, DIM], BF16, kind="ExternalInput").ap()
    wv_ap = nc.dram_tensor("wv", [NCH, 128, DIM], BF16, kind="ExternalInput").ap()
    dw_ap = nc.dram_tensor("dw", [NCH, 128, 9 * 128], BF16, kind="ExternalInput").ap()
    bq_ap = nc.dram_tensor("bq", [128, NCH], F32, kind="ExternalInput").ap()
    bk_ap = nc.dram_tensor("bk", [128, NCH], F32, kind="ExternalInput").ap()
    cb_ap = nc.dram_tensor("cb", [128, NCH], F32, kind="ExternalInput").ap()
    out_ap = nc.dram_tensor("out", [N1, DIM], F32, kind="ExternalOutput").ap()

    with tile.TileContext(nc, trace_sim=trace_sim) as tc:
        if bench_iters:
            with tc.For_i(0, bench_iters, 1):
                _emit(nc, tc, q_ap, k_ap, wq_ap, wk_ap, wv_ap, dw_ap,
                      bq_ap, bk_ap, cb_ap, out_ap, with_cb)
        else:
            _emit(nc, tc, q_ap, k_ap, wq_ap, wk_ap, wv_ap, dw_ap, bq_ap,
                  bk_ap, cb_ap, out_ap, with_cb)
    nc.compile()
    return nc


def _emit(nc, tc, q_ap, k_ap, wq_ap, wk_ap, wv_ap, dw_ap, bq_ap, bk_ap,
          cb_ap, out_ap, with_cb):
    from contextlib import ExitStack
    ctx = ExitStack()

    const = ctx.enter_context(tc.tile_pool(name="const", bufs=1))

    ident_bf = const.tile([128, 128], BF16, tag="identbf", name="identbf")
    make_identity(nc, ident_bf[:])
    ident_f32 = const.tile([128, 128], F32, tag="identf32", name="identf32")
    make_identity(nc, ident_f32[:])
    eps_t = const.tile([128, 1], F32, tag="eps", name="eps")
    nc.gpsimd.memset(eps_t[:], EPS)

    wq_sb = const.tile([128, NCH * DIM], BF16, tag="wq", name="wq")
    wk_sb = const.tile([128, NCH * DIM], BF16, tag="wk", name="wk")
    wv_sb = const.tile([128, NCH * DIM], BF16, tag="wv", name="wv")
    dw_sb = const.tile([128, NCH * 9 * 128], BF16, tag="dw", name="dw")
    bqk_sb = const.tile([128, 3 * NCH], F32, tag="bqk", name="bqk")

    # input staging tiles, 4 token tiles per DMA (batched issues: the HWDGE
    # queue charges a fixed ~625ns per DMA, and DMA_ENGINES is serial)
    xin_k = [const.tile([128, 4 * DIM], F32, tag=f"xk{h}", name=f"xk{h}") for h in range(2)]
    xin_q = [const.tile([128, 4 * DIM], F32, tag=f"xq{h}", name=f"xq{h}") for h in range(2)]

    def in3(t):
        return t[:].rearrange("p (i c) -> p i c", c=DIM)

    # DMA issue order on the sync (HWDGE) queue: k first (LN k leads), each
    # weight just before its first use point.
    k3 = k_ap.rearrange("(i p) c -> p i c", p=128)
    q3 = q_ap.rearrange("(i p) c -> p i c", p=128)
    w3 = {"wk": wk_ap.rearrange("g p c -> p g c"),
          "wq": wq_ap.rearrange("g p c -> p g c"),
          "wv": wv_ap.rearrange("g p c -> p g c"),
          "dw": dw_ap.rearrange("g p c -> p g c")}
    for i in range(NTOK):
        nc.sync.dma_start(in3(xin_k[i // 4])[:, i % 4, :], k3[:, i, :])
    for i in range(NTOK):
        nc.sync.dma_start(in3(xin_q[i // 4])[:, i % 4, :], q3[:, i, :])
    nc.sync.dma_start(wk_sb[:].rearrange("p (g c) -> p g c", c=DIM), w3["wk"])
    nc.sync.dma_start(wq_sb[:].rearrange("p (g c) -> p g c", c=DIM), w3["wq"])
    nc.sync.dma_start(bqk_sb[:, 0:NCH], bq_ap)
    nc.sync.dma_start(bqk_sb[:, NCH:2 * NCH], bk_ap)
    nc.sync.dma_start(bqk_sb[:, 2 * NCH:3 * NCH], cb_ap)
    nc.sync.dma_start(wv_sb[:].rearrange("p (g c) -> p g c", c=DIM), w3["wv"])
    nc.sync.dma_start(dw_sb[:].rearrange("p (g c) -> p g c", c=9 * 128), w3["dw"])

    persist = ctx.enter_context(tc.tile_pool(name="persist", bufs=1))
    lnqT = [persist.tile([128, N1], BF16, tag=f"lnqT{g}", name=f"lnqT{g}") for g in range(NCH)]
    lnkT = [persist.tile([128, N2], BF16, tag=f"lnkT{g}", name=f"lnkT{g}") for g in range(NCH)]
    qT = [persist.tile([128, N1], BF16, tag=f"qT{g}", name=f"qT{g}") for g in range(NCH)]
    kT = [persist.tile([128, N2], BF16, tag=f"kT{g}", name=f"kT{g}") for g in range(NCH)]
    v_aug = [persist.tile([128, NH * (CH + 1)], BF16, tag=f"vaug{i}", name=f"vaug{i}")
             for i in range(NTOK)]
    # token-major skip path: [tok-in-tile, tq tile, channel]
    skip_tok = [persist.tile([128, NTOK * 128], BF16, tag=f"sktok{g}", name=f"sktok{g}")
                for g in range(NCH)]
    # feature-major conv output (pre-transpose); f32 so its transposes can
    # share the conv psum tag (same shape/dtype -> same bank)
    skipT = [persist.tile([128, N1], F32, tag=f"skipT{g}", name=f"skipT{g}")
             for g in range(NCH)]
    # padded conv images, feature-major: [ch, PW*PW]
    qTp = [persist.tile([128, PW * PW], BF16, tag=f"qTp{g}", name=f"qTp{g}")
           for g in range(NCH)]

    ln_state = {}

    def emit_ln_tile(which, i, work, psA, psts, half, last_rstd=None):
        """LN one [128, DIM] input tile and transpose into psts chunks."""
        xt3 = in3((xin_k if which == "k" else xin_q)[i // 4])
        xt = xt3[:, i % 4, :]
        bn6 = work.tile([128, 6], F32, tag="bn6", bufs=8, name="bn6")
        nc.vector.bn_stats(out=bn6[:], in_=xt)
        mv = work.tile([128, 2], F32, tag="mv", bufs=8, name="mv")
        nc.vector.bn_aggr(out=mv[:], in_=bn6[:])
        rstd = work.tile([128, 1], F32, tag="rstd", bufs=8, name="rstd")
        nc.scalar.activation(out=rstd[:], in_=mv[:, 1:2],
                             func=AF.Sqrt, bias=eps_t[:], scale=1.0)
        if last_rstd is not None:
            last_rstd["t"] = rstd
        nc.vector.reciprocal(out=rstd[:], in_=rstd[:])
        nmr = work.tile([128, 1], F32, tag="nmr", bufs=8, name="nmr")
        nc.vector.tensor_scalar(out=nmr[:], in0=mv[:, 0:1],
                                scalar1=rstd[:], scalar2=-1.0,
                                op0=OP.mult, op1=OP.mult)
        ln = work.tile([128, DIM], BF16, tag="ln", bufs=6, name="ln")
        # LN apply on ACT (idle early); the last q tiles on DVE so the ACT
        # serial apply tail doesn't gate the q transposes.
        if which == "q" and i >= 4:
            nc.vector.tensor_scalar(out=ln[:], in0=xt,
                                    scalar1=mv[:, 0:1], scalar2=rstd[:],
                                    op0=OP.subtract, op1=OP.mult)
        elif which == "k":
            nc.gpsimd.tensor_scalar(out=ln[:], in0=xt,
                                    scalar1=mv[:, 0:1], scalar2=rstd[:],
                                    op0=OP.subtract, op1=OP.mult)
        else:
            nc.scalar.activation(out=ln[:], in_=xt, func=AF.Identity,
                                 bias=nmr[:], scale=rstd[:])
        ii = i % 4
        for g in range(NCH):
            nc.tensor.transpose(
                psts[g][:, half * 512 + ii * 128:half * 512 + ii * 128 + 128],
                ln[:, g * 128:(g + 1) * 128],
                ident_bf[:])

    def emit_ln_copyout(which, half, psts):
        # per-half copies so half0 streams out while half1 is in LN.
        # Pool cannot read PSUM, and DVE's in-order queue carries the
        # critical stats chain, so k and q-half0 go to ACT (idle in the
        # prologue) and only q-half1 (after the last stats) to DVE.
        lnT = lnkT if which == "k" else lnqT
        for g in range(NCH):
            sl = slice(half * 512, (half + 1) * 512)
            if which == "q" and half == 1:
                nc.vector.tensor_copy(out=lnT[g][:, sl], in_=psts[g][:, sl])
            else:
                nc.scalar.activation(out=lnT[g][:, sl], in_=psts[g][:, sl],
                                     func=AF.Identity)

    def emit_proj_group(pool, gp, w_sb, bcol, dstT, half, eng=None):
        """One [128(out-feat), 512(tok)] projection psum group + evac."""
        ps = pool.tile([128, 512], F32, tag="proj", name="proj")
        lnT = ln_state["cur_lnT"][(w_sb is wq_sb)]
        for kc in range(NCH):
            nc.tensor.matmul(
                ps[:], w_sb[:, kc * DIM + gp * 128:kc * DIM + (gp + 1) * 128],
                lnT[kc][:, half * 512:(half + 1) * 512],
                start=(kc == 0), stop=(kc == NCH - 1))
        eng = eng or nc.vector
        if eng is nc.scalar:
            nc.scalar.activation(
                out=dstT[gp][:, half * 512:(half + 1) * 512], in_=ps[:],
                func=AF.Identity, bias=bqk_sb[:, bcol + gp:bcol + gp + 1])
        else:
            eng.tensor_scalar(
                out=dstT[gp][:, half * 512:(half + 1) * 512],
                in0=ps[:], scalar1=bqk_sb[:, bcol + gp:bcol + gp + 1],
                scalar2=None, op0=OP.add)

    def emit_v_group(pool, i):
        """v projection for token tile i, token-major, with ones column."""
        nc.gpsimd.tensor_scalar(
            out=v_aug[i][:].rearrange("p (h c) -> p h c", c=CH + 1)[:, :, CH],
            in0=ident_bf[:, 0:NH], scalar1=0.0, scalar2=1.0,
            op0=OP.mult, op1=OP.add)
        ps = pool.tile([128, 512], F32, tag="proj", name="proj")
        for kc in range(NCH):
            nc.tensor.matmul(
                ps[:], lnkT[kc][:, i * 128:(i + 1) * 128],
                wv_sb[:, kc * DIM:(kc + 1) * DIM],
                start=(kc == 0), stop=(kc == NCH - 1))
        nc.vector.tensor_copy(
            out=v_aug[i][:].rearrange("p (h c) -> p h c", c=CH + 1)[:, :, 0:CH],
            in_=ps[:].rearrange("p (h c) -> p h c", c=CH))

    def emit_pads(g):
        """Zero the pad border and copy qT[g] into the padded image."""
        qTp3 = qTp[g][:].rearrange("p (y x) -> p y x", x=PW)
        for view, w in ((qTp3[:, 0, :], PW), (qTp3[:, PW - 1, :], PW),
                        (qTp3[:, 1:PW - 1, 0], PW - 2),
                        (qTp3[:, 1:PW - 1, PW - 1], PW - 2)):
            nc.gpsimd.tensor_scalar(out=view, in0=ident_bf[:, 0:w],
                                    scalar1=0.0, scalar2=None, op0=OP.mult)
        nc.vector.tensor_copy(
            out=qTp3[:, 1:HH + 1, 1:WW + 1],
            in_=qT[g][:].rearrange("p (y x) -> p y x", x=WW))

    def emit_conv_group(psConv, g, yh):
        """Feature-major conv for 16 image rows (=512 px) of chunk g."""
        qTp3 = qTp[g][:].rearrange("p (y x) -> p y x", x=PW)
        dwg = dw_sb[:].rearrange("p (g c) -> p g c", c=9 * 128)
        cps = psConv.tile([128, 512], F32, tag="conv", name="conv")
        ntap = len(TAPS)
        for t, (dy, dx) in enumerate(TAPS):
            y0 = yh * 16 + 1 + dy
            nc.tensor.matmul(
                cps[:],
                dwg[:, g, t * 128:(t + 1) * 128],
                qTp3[:, y0:y0 + 16, 1 + dx:1 + dx + WW],
                start=(t == 0), stop=(t == ntap - 1))
        # bias (conv_b + folded v-bias) rides the evacuation, per-partition
        nc.vector.tensor_scalar(
            out=skipT[g][:, yh * 512:(yh + 1) * 512], in0=cps[:],
            scalar1=bqk_sb[:, 2 * NCH + g:2 * NCH + g + 1], scalar2=None,
            op0=OP.add)

    def emit_skip_transpose(psConv, g, half):
        """Transpose 4 token tiles of skipT[g] into skip_tok[g]."""
        sps = psConv.tile([128, 512], F32, tag="conv", name="skp")
        for i in range(4):
            tb = half * 4 + i
            nc.tensor.transpose(sps[:, i * 128:(i + 1) * 128],
                                skipT[g][:, tb * 128:(tb + 1) * 128],
                                ident_f32[:])
        nc.vector.tensor_copy(
            out=skip_tok[g][:].rearrange("p (i c) -> p i c", c=128)[
                :, half * 4:half * 4 + 4, :],
            in_=sps[:].rearrange("p (i c) -> p i c", c=128))

    def emit_qk(psST, ptP, g, h_loc, j, pts):
        """S^T tile [128 tk, 1024 tq] for head 2g+h_loc, tk chunk j; exp."""
        st = psST.tile([128, N1], F32, tag="st", name="st")
        r = CH * h_loc
        for half in range(2):
            nc.tensor.matmul(
                st[:, half * 512:(half + 1) * 512],
                kT[g][r:r + CH, j * 128:(j + 1) * 128],
                qT[g][r:r + CH, half * 512:(half + 1) * 512],
                start=True, stop=True, tile_position=(r, 0))
        pt = ptP.tile([128, N1], BF16, tag="pt", name="pt")
        nc.scalar.activation(out=pt[:], in_=st[:], func=AF.Exp)
        pts[j] = (pt, 0)

    def emit_pv(psPV, recipP, finP, fins, pts, g, h_loc, tq):
        """x[tq-tile, head 2g+h_loc] in token-major psum; normalize+skip."""
        pv = psPV.tile([128, CH + 1], F32, tag="pv", name="pv")
        h = 2 * g + h_loc
        for j in range(NTOK):
            pt, off = pts[j]
            nc.tensor.matmul(
                pv[:], pt[:, off + tq * 128:off + (tq + 1) * 128],
                v_aug[j][:, h * (CH + 1):(h + 1) * (CH + 1)],
                start=(j == 0), stop=(j == NTOK - 1))
        rc = recipP.tile([128, 1], F32, tag="rc", name="rc")
        nc.vector.reciprocal(out=rc[:], in_=pv[:, CH:CH + 1])
        if h_loc == 0 and tq == 0:
            fins[g] = finP.tile([128, NTOK * 128], F32, tag="fin", name="fin")
        nc.vector.scalar_tensor_tensor(
            out=fins[g][:, tq * 128 + h_loc * CH:tq * 128 + (h_loc + 1) * CH],
            in0=pv[:, 0:CH], scalar=rc[:],
            in1=skip_tok[g][:, tq * 128 + h_loc * CH:tq * 128 + (h_loc + 1) * CH],
            op0=OP.mult, op1=OP.add)
        if h_loc == 1 and tq % 2 == 1 and g == NCH - 1:
            # drain in 2-tq slabs on alternating HWDGE queues
            dq = nc.sync if (tq // 2) % 2 == 0 else nc.scalar
            dq.dma_start(
                out_ap.rearrange("(i p) c -> p i c", p=128)[
                    :, tq - 1:tq + 1, g * 128:(g + 1) * 128],
                fins[g][:].rearrange("p (i c) -> p i c", c=128)[:, tq - 1:tq + 1, :])
        elif h_loc == 1 and tq == NTOK - 1:
            # one column-block DMA per chunk: [8 tq, 128 rows, 128 c]
            nc.sync.dma_start(
                out_ap.rearrange("(i p) c -> p i c", p=128)[:, :, g * 128:(g + 1) * 128],
                fins[g][:].rearrange("p (i c) -> p i c", c=128))

    # ---------------- prologue: LN(k), LN(q) under psA; projP persists ----
    with tc.tile_pool(name="projP", bufs=2, space="PSUM") as projP:
        with tc.tile_pool(name="ln_work", bufs=1) as work, \
             tc.tile_pool(name="psA", bufs=1, space="PSUM") as psA:
            # LN k
            psts = [psA.tile([128, 1024], BF16, tag=f"pst{g}", name=f"pstk{g}")
                    for g in range(NCH)]
            for half in range(2):
                for ii in range(4):
                    emit_ln_tile("k", half * 4 + ii, work, psA, psts, half)
                emit_ln_copyout("k", half, psts)
            ln_state["cur_lnT"] = {False: lnkT, True: lnqT}
            # LN q
            pstsq = [psA.tile([128, 1024], BF16, tag=f"pst{g}", name=f"pstq{g}")
                     for g in range(NCH)]
            last_rstd = {}
            for half in range(2):
                for ii in range(4):
                    emit_ln_tile("q", half * 4 + ii, work, psA, pstsq, half,
                                 last_rstd)
                emit_ln_copyout("q", half, pstsq)
            # dummy exp pinned (via data dep on the last rstd) after the
            # final Sqrt: pulls the exp-table load into the prologue
            dummy = work.tile([128, 1], F32, tag="dummy", name="dummy")
            nc.scalar.activation(out=dummy[:], in_=last_rstd["t"][:],
                                 func=AF.Exp)

        # ------------- main pipelined loop over feature chunks ------------
        pvD = None
        with tc.tile_pool(name="psST", bufs=2, space="PSUM") as psST, \
             tc.tile_pool(name="psPV", bufs=1, space="PSUM") as psPV, \
             tc.tile_pool(name="ptP", bufs=20) as ptP, \
             tc.tile_pool(name="recipP", bufs=6) as recipP, \
             tc.tile_pool(name="finP", bufs=2) as finP:
            psConv_cm = tc.tile_pool(name="psConv", bufs=1, space="PSUM")
            psConv = psConv_cm.__enter__()
            # chunk-0 projections (projP persists across the psA boundary,
            # so these emit after it and don't gate the barrier)
            for half in range(2):
                emit_proj_group(projP, 0, wk_sb, NCH, kT, half, eng=nc.scalar)
                emit_proj_group(projP, 0, wq_sb, 0, qT, half, eng=nc.vector)
            emit_pads(0)
            pts1_prev = None
            fins = {}
            for g in range(NCH):
                if g == NCH - 1:
                    # conv work is fully emitted; swap its bank to a second
                    # pv pool now, while the barrier has nothing to wait on,
                    # so the g=3 PV drain double-buffers under the exp tail.
                    psConv_cm.__exit__(None, None, None)
                    pvD_cm = tc.tile_pool(name="pvD", bufs=1, space="PSUM")
                    pvD = pvD_cm.__enter__()
                # block 1: QK/exp head0; conv(g) for g=0 else PV(g-1, h1)
                pts0 = {}
                for j in range(NTOK):
                    emit_qk(psST, ptP, g, 0, j, pts0)
                    if g == 0:
                        if j % 2 == 0:
                            emit_v_group(projP, j // 2)
                        if j in (0, 2):
                            emit_conv_group(psConv, 0, j // 2)
                        elif j in (4, 6):
                            emit_skip_transpose(psConv, 0, (j - 4) // 2)
                    else:
                        emit_pv(psPV, recipP, finP, fins, pts1_prev,
                                g - 1, 1, j)
                # block 2: QK/exp head1; projections for chunk g+1
                pts1 = {}
                for j in range(NTOK):
                    emit_qk(psST, ptP, g, 1, j, pts1)
                    if g == 0 and j % 2 == 1:
                        emit_v_group(projP, 4 + j // 2)
                    if g < NCH - 1 and j % 2 == 0:
                        jp = j // 2
                        w_sb, bcol, dstT = ((wk_sb, NCH, kT) if jp < 2
                                            else (wq_sb, 0, qT))
                        emit_proj_group(projP, g + 1, w_sb, bcol, dstT,
                                        jp % 2)
                    if g == NCH - 1:
                        emit_pv(psPV if j % 2 == 0 else pvD,
                                recipP, finP, fins, pts0, g, 0, j)
                # block 3: PV head0; conv(g+1); for the last chunk the
                # head1 PV interleaves here so it pipelines under the
                # tail of the exp stream.
                if g < NCH - 1:
                    emit_pads(g + 1)
                    for tq in range(NTOK):
                        emit_pv(psPV, recipP, finP, fins, pts0, g, 0, tq)
                        if tq in (0, 2):
                            emit_conv_group(psConv, g + 1, tq // 2)
                        elif tq in (4, 6):
                            emit_skip_transpose(psConv, g + 1, (tq - 4) // 2)
                else:
                    for tq in range(NTOK):
                        emit_pv(psPV if tq % 2 == 0 else pvD,
                                recipP, finP, fins, pts1, g, 1, tq)
                    pvD_cm.__exit__(None, None, None)
                pts1_prev = pts1

    ctx.close()


_CACHE = {}


def _get_runner(with_cb):
    """Build the program once and wrap it in a reusable jitted SPMD callable."""
    key = ("runner", with_cb)
    if key in _CACHE:
        return _CACHE[key]

    import jax
    from jax.sharding import Mesh, PartitionSpec
    from jax.experimental.shard_map import shard_map
    from concourse import bass2jax
    import concourse.mybir as mb

    nc = _build_program(with_cb=with_cb)
    bass2jax.install_neuronx_cc_hook()

    part_name = (nc.partition_id_tensor.name
                 if nc.partition_id_tensor else None)
    in_names, out_names, out_avals = [], [], []
    for alloc in nc.m.functions[0].allocations:
        if not isinstance(alloc, mb.MemoryLocationSet):
            continue
        name = alloc.memorylocations[0].name
        if alloc.kind == "ExternalInput":
            if name != part_name:
                in_names.append(name)
        elif alloc.kind == "ExternalOutput":
            out_names.append(name)
            out_avals.append(jax.core.ShapedArray(
                tuple(alloc.tensor_shape), mb.dt.np(alloc.dtype)))
    n_params = len(in_names)
    all_names = in_names + out_names
    if part_name is not None:
        all_names = all_names + [part_name]

    def _body(*args):
        operands = list(args)
        if part_name is not None:
            operands.append(bass2jax.partition_id_tensor())
        outs = bass2jax._bass_exec_p.bind(
            *operands,
            out_avals=tuple(out_avals),
            in_names=tuple(all_names),
            out_names=tuple(out_names),
            lowering_input_output_aliases=(),
            sim_require_finite=True,
            sim_require_nnan=True,
            nc=nc,
        )
        return tuple(outs)

    devices = jax.devices()[:N_CORES]
    mesh = Mesh(np.asarray(devices), ("core",))
    n_outs = len(out_names)
    sharded = jax.jit(
        shard_map(_body, mesh=mesh,
                  in_specs=(PartitionSpec("core"),) * (n_params + n_outs),
                  out_specs=(PartitionSpec("core"),) * n_outs,
                  check_rep=False),
        donate_argnums=tuple(range(n_params, n_params + n_outs)),
        keep_unused=True)

    from jax.sharding import NamedSharding
    import jax.numpy as jnp

    zero_shard = NamedSharding(mesh, PartitionSpec("core"))
    make_zeros = jax.jit(
        lambda: tuple(jnp.zeros((N_CORES * a.shape[0], *a.shape[1:]), a.dtype)
                      for a in out_avals),
        out_shardings=(zero_shard,) * len(out_avals))
    dev_cache = {}

    import hashlib

    def run(in_maps):
        concat_in = []
        for name in in_names:
            same = all(in_maps[c][name] is in_maps[0][name]
                       for c in range(N_CORES))
            if same:
                key2 = (name,
                        hashlib.sha1(np.ascontiguousarray(
                            in_maps[0][name]).tobytes()).hexdigest())
                if key2 not in dev_cache:
                    arr = np.concatenate(
                        [np.asarray(in_maps[c][name])
                         for c in range(N_CORES)], axis=0)
                    dev_cache[key2] = jax.device_put(arr, zero_shard)
                concat_in.append(dev_cache[key2])
                continue
            concat_in.append(np.concatenate(
                [np.asarray(in_maps[c][name]) for c in range(N_CORES)],
                axis=0))
        out_arrs = sharded(*concat_in, *make_zeros())
        return [
            {name: np.asarray(out_arrs[i]).reshape(
                N_CORES, *out_avals[i].shape)[c]
             for i, name in enumerate(out_names)}
            for c in range(N_CORES)]

    _CACHE[key] = run
    return run


def _prepare_in_maps(query, key, gq, bq_ln, gk, bk_ln, Wq, bq, Wkv, bkv,
                     conv_w, conv_b, H, W):
    query = np.asarray(query, np.float32)
    key = np.asarray(key, np.float32)
    gq = np.asarray(gq, np.float32); bq_ln = np.asarray(bq_ln, np.float32)
    gk = np.asarray(gk, np.float32); bk_ln = np.asarray(bk_ln, np.float32)
    Wq = np.asarray(Wq, np.float32); bq = np.asarray(bq, np.float32)
    Wkv = np.asarray(Wkv, np.float32); bkv = np.asarray(bkv, np.float32)
    conv_w = np.asarray(conv_w, np.float32)
    conv_b = np.asarray(conv_b, np.float32)
    assert int(H) == HH and int(W) == WW
    B, n1, dim_q = query.shape
    assert (B, n1, dim_q) == (N_CORES, N1, DIM) and key.shape == (N_CORES, N2, DIM)

    scale = (DIM // NH) ** (-0.5)
    # fold LN affine + attention scale into the q projection; the depthwise
    # conv weights absorb the inverse scale (conv is linear in q).
    wq_pre = (gq[:, None] * Wq) * scale
    bq_pre = (bq_ln @ Wq + bq) * scale
    wkv_pre = gk[:, None] * Wkv
    bkv_pre = bk_ln @ Wkv + bkv
    wk_pre, wv_pre = wkv_pre[:, :DIM], wkv_pre[:, DIM:]
    bk_pre, bv_pre = bkv_pre[:DIM], bkv_pre[DIM:]
    # v-bias: softmax weights sum to 1, so +bv on v == +bv on the output;
    # ride it on the conv bias, added via the rank-1 extra conv tap.
    cb_pre = conv_b + bv_pre

    w8 = conv_w[:, 0, :, :] / scale  # [512, 3, 3]
    dw = np.zeros((NCH, 128, 9 * 128), np.float32)
    c = np.arange(128)
    for t, (dy, dx) in enumerate(TAPS):
        wt = w8[:, dy + 1, dx + 1].reshape(NCH, 128)
        for g in range(NCH):
            dw[g, c, t * 128 + c] = wt[g]

    bf = ml_dtypes.bfloat16
    with_cb = False
    common = {
        "wq": np.ascontiguousarray(wq_pre.reshape(NCH, 128, DIM)).astype(bf),
        "wk": np.ascontiguousarray(wk_pre.reshape(NCH, 128, DIM)).astype(bf),
        "wv": np.ascontiguousarray(wv_pre.reshape(NCH, 128, DIM)).astype(bf),
        "dw": dw.astype(bf),
        "bq": np.ascontiguousarray(bq_pre.reshape(NCH, 128).T),
        "bk": np.ascontiguousarray(bk_pre.reshape(NCH, 128).T),
        "cb": np.ascontiguousarray(cb_pre.reshape(NCH, 128).T),
    }
    return with_cb, [dict(common, query=np.ascontiguousarray(query[c]),
                          key=np.ascontiguousarray(key[c]))
                     for c in range(N_CORES)]


def kernel(**inputs):
    with_cb, in_maps = _prepare_in_maps(**inputs)
    run = _get_runner(with_cb)
    results = run(in_maps)
    return np.stack([results[c]["out"] for c in range(N_CORES)], axis=0)


# revision 3
# speedup vs baseline: 1.0014x; 1.0014x over previous
"""Trainium2 Bass kernel for nn_CrossAttention (dense_transformer).

Data-parallel over batch B=8 across the 8 NeuronCores. Per core, a fully
software-pipelined program:

  - LN stats on DVE (bn_stats/bn_aggr), rstd via one ACT Rsqrt, apply split
    DVE/Pool, PE transposes to feature-major bf16.
  - Projections as bf16 PE matmuls (LN affine + 1/sqrt(c) folded on host),
    psum evacuated on Pool (q/k, bias add) and ACT (v, strided token-major).
  - Depthwise 3x3 conv on PE in TOKEN-major orientation: the shifted padded
    image slice is the stationary operand, the 128x128 diagonal weight block
    the moving one, so psum accumulates skip[tok, ch] directly -- no
    transposes of the skip path at all. Bias (zero for this reference) would
    ride as a rank-1 extra tap.
  - Attention transposed (S^T = k^T.T @ q^T) with exp straight out of PSUM
    on ACT; PV in token-major orientation (P^T stationary, v_aug moving,
    65-wide free dim) so x[tok, c] + rowsum land in psum with no final
    transpose. Row sums via the ones-column-in-V trick.
  - Final normalize + skip add as one scalar_tensor_tensor per (tq, head)
    on Pool, reciprocals on DVE, column-block DMA out.

Emission interleaves chunk g's attention with chunk g+1's projections and
conv so the PE stream never waits on the ACT exp stream (the two are within
~10% of each other); PSUM peaks at exactly 8 banks.
"""

import numpy as np
import ml_dtypes

import concourse.bass as bass
import concourse.mybir as mybir
import concourse.tile as tile
from concourse import bacc, bass_utils
from concourse.masks import make_identity

F32 = mybir.dt.float32
BF16 = mybir.dt.bfloat16
AF = mybir.ActivationFunctionType
OP = mybir.AluOpType

N_CORES = 8
N1 = 1024          # query tokens (= H*W = 32*32)
N2 = 1024          # key tokens
DIM = 512
NH = 8
CH = 64            # head dim
HH = 32            # H
WW = 32            # W
NTOK = N1 // 128   # 8 token tiles
NCH = DIM // 128   # 4 feature chunks
EPS = 1e-5
PW = WW + 2        # padded image width

# tap order: center first so its start=True write covers the full psum
# region before the edge taps accumulate.
TAPS = [(0, 0), (-1, -1), (-1, 0), (-1, 1), (0, -1), (0, 1),
        (1, -1), (1, 0), (1, 1)]


def _build_program(trace_sim=False, bench_iters=0, with_cb=False):
    nc = bacc.Bacc("TRN2", target_bir_lowering=False, debug=False,
                   enable_asserts=True, num_devices=N_CORES)

    q_ap = nc.dram_tensor("query", [N1, DIM], F32, kind="ExternalInput").ap()
    k_ap = nc.dram_tensor("key", [N2, DIM], F32, kind="ExternalInput").ap()
    wq_ap = nc.dram_tensor("wq", [NCH, 128, DIM], BF16, kind="ExternalInput").ap()
    wk_ap = nc.dram_tensor("wk", [NCH, 128, DIM], BF16, kind="ExternalInput").ap()
    wv_ap = nc.dram_tensor("wv", [NCH, 128, DIM], BF16, kind="ExternalInput").ap()
    dw_ap = nc.dram_tensor("dw", [NCH, 128, 9 * 128], BF16, kind="ExternalInput").ap()
    bq_ap = nc.dram_tensor("bq", [128, NCH], F32, kind="ExternalInput").ap()
    bk_ap = nc.dram_tensor("bk", [128, NCH], F32, kind="ExternalInput").ap()
    cb_ap = nc.dram_tensor("cb", [128, NCH], F32, kind="ExternalInput").ap()
    out_ap = nc.dram_tensor("out", [N1, DIM], F32, kind="ExternalOutput").ap()

    with tile.TileContext(nc, trace_sim=trace_sim) as tc:
        if bench_iters:
            with tc.For_i(0, bench_iters, 1):
                _emit(nc, tc, q_ap, k_ap, wq_ap, wk_ap, wv_ap, dw_ap,
                      bq_ap, bk_ap, cb_ap, out_ap, with_cb)
        else:
            _emit(nc, tc, q_ap, k_ap, wq_ap, wk_ap, wv_ap, dw_ap, bq_ap,
                  bk_ap, cb_ap, out_ap, with_cb)
    nc.compile()
    return nc


def _emit(nc, tc, q_ap, k_ap, wq_ap, wk_ap, wv_ap, dw_ap, bq_ap, bk_ap,
          cb_ap, out_ap, with_cb):
    from contextlib import ExitStack
    ctx = ExitStack()

    const = ctx.enter_context(tc.tile_pool(name="const", bufs=1))

    ident_bf = const.tile([128, 128], BF16, tag="identbf", name="identbf")
    make_identity(nc, ident_bf[:])
    ident_f32 = const.tile([128, 128], F32, tag="identf32", name="identf32")
    make_identity(nc, ident_f32[:])
    eps_t = const.tile([128, 1], F32, tag="eps", name="eps")
    nc.gpsimd.memset(eps_t[:], EPS)

    wq_sb = const.tile([128, NCH * DIM], BF16, tag="wq", name="wq")
    wk_sb = const.tile([128, NCH * DIM], BF16, tag="wk", name="wk")
    wv_sb = const.tile([128, NCH * DIM], BF16, tag="wv", name="wv")
    dw_sb = const.tile([128, NCH * 9 * 128], BF16, tag="dw", name="dw")
    bqk_sb = const.tile([128, 3 * NCH], F32, tag="bqk", name="bqk")

    # input staging tiles, 4 token tiles per DMA (batched issues: the HWDGE
    # queue charges a fixed ~625ns per DMA, and DMA_ENGINES is serial)
    xin_k = [const.tile([128, 4 * DIM], F32, tag=f"xk{h}", name=f"xk{h}") for h in range(2)]
    xin_q = [const.tile([128, 4 * DIM], F32, tag=f"xq{h}", name=f"xq{h}") for h in range(2)]

    def in3(t):
        return t[:].rearrange("p (i c) -> p i c", c=DIM)

    # DMA issue order on the sync (HWDGE) queue: k first (LN k leads), each
    # weight just before its first use point.
    k3 = k_ap.rearrange("(i p) c -> p i c", p=128)
    q3 = q_ap.rearrange("(i p) c -> p i c", p=128)
    w3 = {"wk": wk_ap.rearrange("g p c -> p g c"),
          "wq": wq_ap.rearrange("g p c -> p g c"),
          "wv": wv_ap.rearrange("g p c -> p g c"),
          "dw": dw_ap.rearrange("g p c -> p g c")}
    for i in range(NTOK):
        nc.sync.dma_start(in3(xin_k[i // 4])[:, i % 4, :], k3[:, i, :])
    for i in range(NTOK):
        nc.sync.dma_start(in3(xin_q[i // 4])[:, i % 4, :], q3[:, i, :])
    nc.sync.dma_start(wk_sb[:].rearrange("p (g c) -> p g c", c=DIM), w3["wk"])
    nc.sync.dma_start(wq_sb[:].rearrange("p (g c) -> p g c", c=DIM), w3["wq"])
    nc.sync.dma_start(bqk_sb[:, 0:NCH], bq_ap)
    nc.sync.dma_start(bqk_sb[:, NCH:2 * NCH], bk_ap)
    nc.sync.dma_start(bqk_sb[:, 2 * NCH:3 * NCH], cb_ap)
    nc.sync.dma_start(wv_sb[:].rearrange("p (g c) -> p g c", c=DIM), w3["wv"])
    nc.sync.dma_start(dw_sb[:].rearrange("p (g c) -> p g c", c=9 * 128), w3["dw"])

    persist = ctx.enter_context(tc.tile_pool(name="persist", bufs=1))
    lnqT = [persist.tile([128, N1], BF16, tag=f"lnqT{g}", name=f"lnqT{g}") for g in range(NCH)]
    lnkT = [persist.tile([128, N2], BF16, tag=f"lnkT{g}", name=f"lnkT{g}") for g in range(NCH)]
    qT = [persist.tile([128, N1], BF16, tag=f"qT{g}", name=f"qT{g}") for g in range(NCH)]
    kT = [persist.tile([128, N2], BF16, tag=f"kT{g}", name=f"kT{g}") for g in range(NCH)]
    v_aug = [persist.tile([128, NH * (CH + 1)], BF16, tag=f"vaug{i}", name=f"vaug{i}")
             for i in range(NTOK)]
    # token-major skip path: [tok-in-tile, tq tile, channel]
    skip_tok = [persist.tile([128, NTOK * 128], BF16, tag=f"sktok{g}", name=f"sktok{g}")
                for g in range(NCH)]
    # feature-major conv output (pre-transpose); f32 so its transposes can
    # share the conv psum tag (same shape/dtype -> same bank)
    skipT = [persist.tile([128, N1], F32, tag=f"skipT{g}", name=f"skipT{g}")
             for g in range(NCH)]
    # padded conv images, feature-major: [ch, PW*PW]
    qTp = [persist.tile([128, PW * PW], BF16, tag=f"qTp{g}", name=f"qTp{g}")
           for g in range(NCH)]

    ln_state = {}

    def emit_ln_tile(which, i, work, psA, psts, half, last_rstd=None):
        """LN one [128, DIM] input tile and transpose into psts chunks."""
        xt3 = in3((xin_k if which == "k" else xin_q)[i // 4])
        xt = xt3[:, i % 4, :]
        bn6 = work.tile([128, 6], F32, tag="bn6", bufs=16, name="bn6")
        nc.vector.bn_stats(out=bn6[:], in_=xt)
        mv = work.tile([128, 2], F32, tag="mv", bufs=16, name="mv")
        nc.vector.bn_aggr(out=mv[:], in_=bn6[:])
        rstd = work.tile([128, 1], F32, tag="rstd", bufs=16, name="rstd")
        nc.scalar.activation(out=rstd[:], in_=mv[:, 1:2],
                             func=AF.Sqrt, bias=eps_t[:], scale=1.0)
        if last_rstd is not None:
            last_rstd["t"] = rstd
        nc.vector.reciprocal(out=rstd[:], in_=rstd[:])
        nmr = work.tile([128, 1], F32, tag="nmr", bufs=16, name="nmr")
        nc.vector.tensor_scalar(out=nmr[:], in0=mv[:, 0:1],
                                scalar1=rstd[:], scalar2=-1.0,
                                op0=OP.mult, op1=OP.mult)
        ln = work.tile([128, DIM], BF16, tag="ln", bufs=10, name="ln")
        # LN apply on ACT (idle early); the last q tiles on DVE so the ACT
        # serial apply tail doesn't gate the q transposes.
        if which == "q" and i >= 4:
            nc.vector.tensor_scalar(out=ln[:], in0=xt,
                                    scalar1=mv[:, 0:1], scalar2=rstd[:],
                                    op0=OP.subtract, op1=OP.mult)
        elif which == "k":
            nc.gpsimd.tensor_scalar(out=ln[:], in0=xt,
                                    scalar1=mv[:, 0:1], scalar2=rstd[:],
                                    op0=OP.subtract, op1=OP.mult)
        else:
            nc.scalar.activation(out=ln[:], in_=xt, func=AF.Identity,
                                 bias=nmr[:], scale=rstd[:])
        ii = i % 4
        for g in range(NCH):
            nc.tensor.transpose(
                psts[g][:, half * 512 + ii * 128:half * 512 + ii * 128 + 128],
                ln[:, g * 128:(g + 1) * 128],
                ident_bf[:])

    def emit_ln_copyout(which, half, psts):
        # per-half copies so half0 streams out while half1 is in LN.
        # Pool cannot read PSUM, and DVE's in-order queue carries the
        # critical stats chain, so k and q-half0 go to ACT (idle in the
        # prologue) and only q-half1 (after the last stats) to DVE.
        lnT = lnkT if which == "k" else lnqT
        for g in range(NCH):
            sl = slice(half * 512, (half + 1) * 512)
            if which == "q" and half == 1:
                nc.vector.tensor_copy(out=lnT[g][:, sl], in_=psts[g][:, sl])
            else:
                nc.scalar.activation(out=lnT[g][:, sl], in_=psts[g][:, sl],
                                     func=AF.Identity)

    def emit_proj_group(pool, gp, w_sb, bcol, dstT, half, eng=None):
        """One [128(out-feat), 512(tok)] projection psum group + evac."""
        ps = pool.tile([128, 512], F32, tag="proj", name="proj")
        lnT = ln_state["cur_lnT"][(w_sb is wq_sb)]
        for kc in range(NCH):
            nc.tensor.matmul(
                ps[:], w_sb[:, kc * DIM + gp * 128:kc * DIM + (gp + 1) * 128],
                lnT[kc][:, half * 512:(half + 1) * 512],
                start=(kc == 0), stop=(kc == NCH - 1))
        eng = eng or nc.vector
        if eng is nc.scalar:
            nc.scalar.activation(
                out=dstT[gp][:, half * 512:(half + 1) * 512], in_=ps[:],
                func=AF.Identity, bias=bqk_sb[:, bcol + gp:bcol + gp + 1])
        else:
            eng.tensor_scalar(
                out=dstT[gp][:, half * 512:(half + 1) * 512],
                in0=ps[:], scalar1=bqk_sb[:, bcol + gp:bcol + gp + 1],
                scalar2=None, op0=OP.add)

    def emit_v_group(pool, i):
        """v projection for token tile i, token-major, with ones column."""
        nc.gpsimd.tensor_scalar(
            out=v_aug[i][:].rearrange("p (h c) -> p h c", c=CH + 1)[:, :, CH],
            in0=ident_bf[:, 0:NH], scalar1=0.0, scalar2=1.0,
            op0=OP.mult, op1=OP.add)
        ps = pool.tile([128, 512], F32, tag="proj", name="proj")
        for kc in range(NCH):
            nc.tensor.matmul(
                ps[:], lnkT[kc][:, i * 128:(i + 1) * 128],
                wv_sb[:, kc * DIM:(kc + 1) * DIM],
                start=(kc == 0), stop=(kc == NCH - 1))
        nc.vector.tensor_copy(
            out=v_aug[i][:].rearrange("p (h c) -> p h c", c=CH + 1)[:, :, 0:CH],
            in_=ps[:].rearrange("p (h c) -> p h c", c=CH))

    def emit_pads(g):
        """Zero the pad border and copy qT[g] into the padded image."""
        qTp3 = qTp[g][:].rearrange("p (y x) -> p y x", x=PW)
        for view, w in ((qTp3[:, 0, :], PW), (qTp3[:, PW - 1, :], PW),
                        (qTp3[:, 1:PW - 1, 0], PW - 2),
                        (qTp3[:, 1:PW - 1, PW - 1], PW - 2)):
            nc.gpsimd.tensor_scalar(out=view, in0=ident_bf[:, 0:w],
                                    scalar1=0.0, scalar2=None, op0=OP.mult)
        nc.vector.tensor_copy(
            out=qTp3[:, 1:HH + 1, 1:WW + 1],
            in_=qT[g][:].rearrange("p (y x) -> p y x", x=WW))

    def emit_conv_group(psConv, g, yh):
        """Feature-major conv for 16 image rows (=512 px) of chunk g."""
        qTp3 = qTp[g][:].rearrange("p (y x) -> p y x", x=PW)
        dwg = dw_sb[:].rearrange("p (g c) -> p g c", c=9 * 128)
        cps = psConv.tile([128, 512], F32, tag="conv", name="conv")
        ntap = len(TAPS)
        for t, (dy, dx) in enumerate(TAPS):
            y0 = yh * 16 + 1 + dy
            nc.tensor.matmul(
                cps[:],
                dwg[:, g, t * 128:(t + 1) * 128],
                qTp3[:, y0:y0 + 16, 1 + dx:1 + dx + WW],
                start=(t == 0), stop=(t == ntap - 1))
        # bias (conv_b + folded v-bias) rides the evacuation, per-partition
        nc.vector.tensor_scalar(
            out=skipT[g][:, yh * 512:(yh + 1) * 512], in0=cps[:],
            scalar1=bqk_sb[:, 2 * NCH + g:2 * NCH + g + 1], scalar2=None,
            op0=OP.add)

    def emit_skip_transpose(psConv, g, half):
        """Transpose 4 token tiles of skipT[g] into skip_tok[g]."""
        sps = psConv.tile([128, 512], F32, tag="conv", name="skp")
        for i in range(4):
            tb = half * 4 + i
            nc.tensor.transpose(sps[:, i * 128:(i + 1) * 128],
                                skipT[g][:, tb * 128:(tb + 1) * 128],
                                ident_f32[:])
        nc.vector.tensor_copy(
            out=skip_tok[g][:].rearrange("p (i c) -> p i c", c=128)[
                :, half * 4:half * 4 + 4, :],
            in_=sps[:].rearrange("p (i c) -> p i c", c=128))

    def emit_qk(psST, ptP, g, h_loc, j, pts):
        """S^T tile [128 tk, 1024 tq] for head 2g+h_loc, tk chunk j; exp."""
        st = psST.tile([128, N1], F32, tag="st", name="st")
        r = CH * h_loc
        for half in range(2):
            nc.tensor.matmul(
                st[:, half * 512:(half + 1) * 512],
                kT[g][r:r + CH, j * 128:(j + 1) * 128],
                qT[g][r:r + CH, half * 512:(half + 1) * 512],
                start=True, stop=True, tile_position=(r, 0))
        pt = ptP.tile([128, N1], BF16, tag="pt", name="pt")
        nc.scalar.activation(out=pt[:], in_=st[:], func=AF.Exp)
        pts[j] = (pt, 0)

    def emit_pv(psPV, recipP, finP, fins, pts, g, h_loc, tq):
        """x[tq-tile, head 2g+h_loc] in token-major psum; normalize+skip."""
        pv = psPV.tile([128, CH + 1], F32, tag="pv", name="pv")
        h = 2 * g + h_loc
        for j in range(NTOK):
            pt, off = pts[j]
            nc.tensor.matmul(
                pv[:], pt[:, off + tq * 128:off + (tq + 1) * 128],
                v_aug[j][:, h * (CH + 1):(h + 1) * (CH + 1)],
                start=(j == 0), stop=(j == NTOK - 1))
        rc = recipP.tile([128, 1], F32, tag="rc", name="rc")
        nc.vector.reciprocal(out=rc[:], in_=pv[:, CH:CH + 1])
        if h_loc == 0 and tq == 0:
            fins[g] = finP.tile([128, NTOK * 128], F32, tag="fin", name="fin")
        nc.vector.scalar_tensor_tensor(
            out=fins[g][:, tq * 128 + h_loc * CH:tq * 128 + (h_loc + 1) * CH],
            in0=pv[:, 0:CH], scalar=rc[:],
            in1=skip_tok[g][:, tq * 128 + h_loc * CH:tq * 128 + (h_loc + 1) * CH],
            op0=OP.mult, op1=OP.add)
        if h_loc == 1 and tq % 2 == 1 and g == NCH - 1:
            # drain in 2-tq slabs on alternating HWDGE queues
            dq = nc.sync if (tq // 2) % 2 == 0 else nc.scalar
            dq.dma_start(
                out_ap.rearrange("(i p) c -> p i c", p=128)[
                    :, tq - 1:tq + 1, g * 128:(g + 1) * 128],
                fins[g][:].rearrange("p (i c) -> p i c", c=128)[:, tq - 1:tq + 1, :])
        elif h_loc == 1 and tq == NTOK - 1:
            # one column-block DMA per chunk: [8 tq, 128 rows, 128 c]
            nc.sync.dma_start(
                out_ap.rearrange("(i p) c -> p i c", p=128)[:, :, g * 128:(g + 1) * 128],
                fins[g][:].rearrange("p (i c) -> p i c", c=128))

    # ---------------- prologue: LN(k), LN(q) under psA; projP persists ----
    with tc.tile_pool(name="projP", bufs=2, space="PSUM") as projP:
        with tc.tile_pool(name="ln_work", bufs=1) as work, \
             tc.tile_pool(name="psA", bufs=1, space="PSUM") as psA:
            # LN k
            psts = [psA.tile([128, 1024], BF16, tag=f"pst{g}", name=f"pstk{g}")
                    for g in range(NCH)]
            for half in range(2):
                for ii in range(4):
                    emit_ln_tile("k", half * 4 + ii, work, psA, psts, half)
                emit_ln_copyout("k", half, psts)
            ln_state["cur_lnT"] = {False: lnkT, True: lnqT}
            # LN q
            pstsq = [psA.tile([128, 1024], BF16, tag=f"pst{g}", name=f"pstq{g}")
                     for g in range(NCH)]
            last_rstd = {}
            for half in range(2):
                for ii in range(4):
                    emit_ln_tile("q", half * 4 + ii, work, psA, pstsq, half,
                                 last_rstd)
                emit_ln_copyout("q", half, pstsq)
            # dummy exp pinned (via data dep on the last rstd) after the
            # final Sqrt: pulls the exp-table load into the prologue
            dummy = work.tile([128, 1], F32, tag="dummy", name="dummy")
            nc.scalar.activation(out=dummy[:], in_=last_rstd["t"][:],
                                 func=AF.Exp)

        # ------------- main pipelined loop over feature chunks ------------
        pvD = None
        with tc.tile_pool(name="psST", bufs=2, space="PSUM") as psST, \
             tc.tile_pool(name="psPV", bufs=1, space="PSUM") as psPV, \
             tc.tile_pool(name="ptP", bufs=26) as ptP, \
             tc.tile_pool(name="recipP", bufs=12) as recipP, \
             tc.tile_pool(name="finP", bufs=3) as finP:
            psConv_cm = tc.tile_pool(name="psConv", bufs=1, space="PSUM")
            psConv = psConv_cm.__enter__()
            # chunk-0 projections (projP persists across the psA boundary,
            # so these emit after it and don't gate the barrier)
            for half in range(2):
                emit_proj_group(projP, 0, wk_sb, NCH, kT, half, eng=nc.scalar)
                emit_proj_group(projP, 0, wq_sb, 0, qT, half, eng=nc.vector)
            emit_pads(0)
            pts1_prev = None
            fins = {}
            for g in range(NCH):
                if g == NCH - 1:
                    # conv work is fully emitted; swap its bank to a second
                    # pv pool now, while the barrier has nothing to wait on,
                    # so the g=3 PV drain double-buffers under the exp tail.
                    psConv_cm.__exit__(None, None, None)
                    pvD_cm = tc.tile_pool(name="pvD", bufs=1, space="PSUM")
                    pvD = pvD_cm.__enter__()
                # block 1: QK/exp head0; conv(g) for g=0 else PV(g-1, h1)
                pts0 = {}
                for j in range(NTOK):
                    emit_qk(psST, ptP, g, 0, j, pts0)
                    if g == 0:
                        if j % 2 == 0:
                            emit_v_group(projP, j // 2)
                        if j in (0, 2):
                            emit_conv_group(psConv, 0, j // 2)
                        elif j in (4, 6):
                            emit_skip_transpose(psConv, 0, (j - 4) // 2)
                    else:
                        emit_pv(psPV, recipP, finP, fins, pts1_prev,
                                g - 1, 1, j)
                # block 2: QK/exp head1; projections for chunk g+1
                pts1 = {}
                for j in range(NTOK):
                    emit_qk(psST, ptP, g, 1, j, pts1)
                    if g == 0 and j % 2 == 1:
                        emit_v_group(projP, 4 + j // 2)
                    if g < NCH - 1 and j % 2 == 0:
                        jp = j // 2
                        w_sb, bcol, dstT = ((wk_sb, NCH, kT) if jp < 2
                                            else (wq_sb, 0, qT))
                        emit_proj_group(projP, g + 1, w_sb, bcol, dstT,
                                        jp % 2)
                    if g == NCH - 1:
                        emit_pv(psPV if j % 2 == 0 else pvD,
                                recipP, finP, fins, pts0, g, 0, j)
                # block 3: PV head0; conv(g+1); for the last chunk the
                # head1 PV interleaves here so it pipelines under the
                # tail of the exp stream.
                if g < NCH - 1:
                    emit_pads(g + 1)
                    for tq in range(NTOK):
                        emit_pv(psPV, recipP, finP, fins, pts0, g, 0, tq)
                        if tq in (0, 2):
                            emit_conv_group(psConv, g + 1, tq // 2)
                        elif tq in (4, 6):
                            emit_skip_transpose(psConv, g + 1, (tq - 4) // 2)
                else:
                    for tq in range(NTOK):
                        emit_pv(psPV if tq % 2 == 0 else pvD,
                                recipP, finP, fins, pts1, g, 1, tq)
                    pvD_cm.__exit__(None, None, None)
                pts1_prev = pts1

    ctx.close()


_CACHE = {}


def _get_runner(with_cb):
    """Build the program once and wrap it in a reusable jitted SPMD callable."""
    key = ("runner", with_cb)
    if key in _CACHE:
        return _CACHE[key]

    import jax
    from jax.sharding import Mesh, PartitionSpec
    from jax.experimental.shard_map import shard_map
    from concourse import bass2jax
    import concourse.mybir as mb

    nc = _build_program(with_cb=with_cb)
    bass2jax.install_neuronx_cc_hook()

    part_name = (nc.partition_id_tensor.name
                 if nc.partition_id_tensor else None)
    in_names, out_names, out_avals = [], [], []
    for alloc in nc.m.functions[0].allocations:
        if not isinstance(alloc, mb.MemoryLocationSet):
            continue
        name = alloc.memorylocations[0].name
        if alloc.kind == "ExternalInput":
            if name != part_name:
                in_names.append(name)
        elif alloc.kind == "ExternalOutput":
            out_names.append(name)
            out_avals.append(jax.core.ShapedArray(
                tuple(alloc.tensor_shape), mb.dt.np(alloc.dtype)))
    n_params = len(in_names)
    all_names = in_names + out_names
    if part_name is not None:
        all_names = all_names + [part_name]

    def _body(*args):
        operands = list(args)
        if part_name is not None:
            operands.append(bass2jax.partition_id_tensor())
        outs = bass2jax._bass_exec_p.bind(
            *operands,
            out_avals=tuple(out_avals),
            in_names=tuple(all_names),
            out_names=tuple(out_names),
            lowering_input_output_aliases=(),
            sim_require_finite=True,
            sim_require_nnan=True,
            nc=nc,
        )
        return tuple(outs)

    devices = jax.devices()[:N_CORES]
    mesh = Mesh(np.asarray(devices), ("core",))
    n_outs = len(out_names)
    sharded = jax.jit(
        shard_map(_body, mesh=mesh,
                  in_specs=(PartitionSpec("core"),) * (n_params + n_outs),
                  out_specs=(PartitionSpec("core"),) * n_outs,
                  check_rep=False),
        donate_argnums=tuple(range(n_params, n_params + n_outs)),
        keep_unused=True)

    from jax.sharding import NamedSharding
    import jax.numpy as jnp

    zero_shard = NamedSharding(mesh, PartitionSpec("core"))
    make_zeros = jax.jit(
        lambda: tuple(jnp.zeros((N_CORES * a.shape[0], *a.shape[1:]), a.dtype)
                      for a in out_avals),
        out_shardings=(zero_shard,) * len(out_avals))
    dev_cache = {}

    import hashlib

    def run(in_maps):
        concat_in = []
        for name in in_names:
            same = all(in_maps[c][name] is in_maps[0][name]
                       for c in range(N_CORES))
            if same:
                key2 = (name,
                        hashlib.sha1(np.ascontiguousarray(
                            in_maps[0][name]).tobytes()).hexdigest())
                if key2 not in dev_cache:
                    arr = np.concatenate(
                        [np.asarray(in_maps[c][name])
                         for c in range(N_CORES)], axis=0)
                    dev_cache[key2] = jax.device_put(arr, zero_shard)
                concat_in.append(dev_cache[key2])
                continue
            concat_in.append(np.concatenate(
                [np.asarray(in_maps[c][name]) for c in range(N_CORES)],
                axis=0))
        out_arrs = sharded(*concat_in, *make_zeros())
        return [
            {name: np.asarray(out_arrs[i]).reshape(
                N_CORES, *out_avals[i].shape)[c]
             for i, name in enumerate(out_names)}
            for c in range(N_CORES)]

    _CACHE[key] = run
    return run


def _prepare_in_maps(query, key, gq, bq_ln, gk, bk_ln, Wq, bq, Wkv, bkv,
                     conv_w, conv_b, H, W):
    query = np.asarray(query, np.float32)
    key = np.asarray(key, np.float32)
    gq = np.asarray(gq, np.float32); bq_ln = np.asarray(bq_ln, np.float32)
    gk = np.asarray(gk, np.float32); bk_ln = np.asarray(bk_ln, np.float32)
    Wq = np.asarray(Wq, np.float32); bq = np.asarray(bq, np.float32)
    Wkv = np.asarray(Wkv, np.float32); bkv = np.asarray(bkv, np.float32)
    conv_w = np.asarray(conv_w, np.float32)
    conv_b = np.asarray(conv_b, np.float32)
    assert int(H) == HH and int(W) == WW
    B, n1, dim_q = query.shape
    assert (B, n1, dim_q) == (N_CORES, N1, DIM) and key.shape == (N_CORES, N2, DIM)

    scale = (DIM // NH) ** (-0.5)
    # fold LN affine + attention scale into the q projection; the depthwise
    # conv weights absorb the inverse scale (conv is linear in q).
    wq_pre = (gq[:, None] * Wq) * scale
    bq_pre = (bq_ln @ Wq + bq) * scale
    wkv_pre = gk[:, None] * Wkv
    bkv_pre = bk_ln @ Wkv + bkv
    wk_pre, wv_pre = wkv_pre[:, :DIM], wkv_pre[:, DIM:]
    bk_pre, bv_pre = bkv_pre[:DIM], bkv_pre[DIM:]
    # v-bias: softmax weights sum to 1, so +bv on v == +bv on the output;
    # ride it on the conv bias, added via the rank-1 extra conv tap.
    cb_pre = conv_b + bv_pre

    w8 = conv_w[:, 0, :, :] / scale  # [512, 3, 3]
    dw = np.zeros((NCH, 128, 9 * 128), np.float32)
    c = np.arange(128)
    for t, (dy, dx) in enumerate(TAPS):
        wt = w8[:, dy + 1, dx + 1].reshape(NCH, 128)
        for g in range(NCH):
            dw[g, c, t * 128 + c] = wt[g]

    bf = ml_dtypes.bfloat16
    with_cb = False
    common = {
        "wq": np.ascontiguousarray(wq_pre.reshape(NCH, 128, DIM)).astype(bf),
        "wk": np.ascontiguousarray(wk_pre.reshape(NCH, 128, DIM)).astype(bf),
        "wv": np.ascontiguousarray(wv_pre.reshape(NCH, 128, DIM)).astype(bf),
        "dw": dw.astype(bf),
        "bq": np.ascontiguousarray(bq_pre.reshape(NCH, 128).T),
        "bk": np.ascontiguousarray(bk_pre.reshape(NCH, 128).T),
        "cb": np.ascontiguousarray(cb_pre.reshape(NCH, 128).T),
    }
    return with_cb, [dict(common, query=np.ascontiguousarray(query[c]),
                          key=np.ascontiguousarray(key[c]))
                     for c in range(N_CORES)]


def kernel(**inputs):
    with_cb, in_maps = _prepare_in_maps(**inputs)
    run = _get_runner(with_cb)
    results = run(in_maps)
    return np.stack([results[c]["out"] for c in range(N_CORES)], axis=0)
